# revision 3
# baseline (speedup 1.0000x reference)
"""DualTowerGCN Trainium2 kernel.

Strategy (8 NeuronCores):
  - Model-parallel across towers: cores 0-3 run tower1, cores 4-7 run tower2
    (tower2's 64-dim input is zero-padded to 128 so one SPMD program serves
    both groups).
  - Within a tower, destinations are partitioned 4 ways (12500 nodes/core).
    Each GCN conv is a gather + one-hot-matmul scatter-add:
      * edges (incl. self loops, with symmetric-norm coefficients folded in)
        are sorted by destination tile (128 dests) and split by source half
        (dma_gather indices are int16, so the 50000-row table is addressed as
        two <=32768-row views);
      * dma_gather pulls 512B source rows in 4096-row batches;
      * per 128-edge chunk, a single dual-op DVE tensor_scalar builds
        onehot[e,d] = (iota==col[e]) * norm[e]; PE accumulates
        psum[f,d] += gathered[e,f].T @ onehot[e,d] over the tile's chunks;
      * the weight matrix is applied after the scatter (matmul associativity),
        bias via a rank-1 matmul accumulate.
  - One AllGather (groups [0-3], [4-7]) shares conv1 outputs for conv2.
  - Pairwise-max pooling (cluster = arange//2 in this workload's graclus
    stand-in) is a strided free-dim max; FC + sigmoid finish on device.
"""

import sys

sys.path.insert(0, "/opt/trn_rl_repo")

import numpy as np

import concourse.bacc as bacc
import concourse.tile as tile
from concourse import mybir
from concourse.bass_utils import run_bass_kernel_spmd

P = 128
N = 50000
E = 600000
F = 128  # feature width (tower2 padded 64 -> 128)
NCORES = 8
CPT = 4  # cores per tower
DPC = N // CPT  # dests per core = 12500
NT = (DPC + P - 1) // P  # dest tiles per core = 98
LAST_DV = DPC - (NT - 1) * P  # valid dests in last tile = 84
SPLIT = 32768  # int16 gather-index limit
GCH = 32  # chunks per dma_gather (4096 rows)

LAST_EXEC_NS = None


# ---------------------------------------------------------------- host prep
def _edge_streams(x_rows, ei, ew):
    """Per-tower edge preprocessing: add self loops, compute GCN symmetric
    norms, partition by dest core, sort by dest tile, split by source half.

    Returns per-core dicts of per-(tile, stream) edge lists."""
    row = np.asarray(ei[0], dtype=np.int64)
    col = np.asarray(ei[1], dtype=np.int64)
    w = np.asarray(ew, dtype=np.float32)

    deg = np.zeros(N, np.float32)
    np.add.at(deg, col, w)
    deg += 1.0  # self loops
    dinv = (1.0 / np.sqrt(deg)).astype(np.float32)

    rows = np.concatenate([row, np.arange(N, dtype=np.int64)])
    cols = np.concatenate([col, np.arange(N, dtype=np.int64)])
    norms = np.concatenate([dinv[row] * w * dinv[col], dinv * dinv]).astype(np.float32)

    cores = []
    for pcore in range(CPT):
        m = (cols // DPC) == pcore
        r, c, nr = rows[m], cols[m] - pcore * DPC, norms[m]
        t = c // P
        half = (r >= SPLIT).astype(np.int64)
        key = t * 2 + half
        order = np.argsort(key, kind="stable")
        r, c, nr, key = r[order], c[order], nr[order], key[order]
        cnt = np.bincount(key, minlength=NT * 2)
        off = np.concatenate([[0], np.cumsum(cnt)])
        cores.append({"r": r, "c": c % P, "n": nr, "cnt": cnt, "off": off})
    return cores


def _pack_core(core, nA, nB):
    """Lay one core's edges into the program-uniform chunk layout.

    Returns srcA/srcB (int16 flat, chunk-major), colA/colB + nrmA/nrmB
    ([128, S] f32)."""
    outs = []
    for half, nprog in ((0, nA), (1, nB)):
        S = int(nprog.sum())
        src = np.zeros(S * P, np.int16)
        colf = np.zeros(S * P, np.float32)
        nrm = np.zeros(S * P, np.float32)
        cursor = 0
        for t in range(NT):
            k = t * 2 + half
            a, b = core["off"][k], core["off"][k + 1]
            cntk = b - a
            span = int(nprog[t]) * P
            assert cntk <= span
            r = core["r"][a:b]
            if half:
                r = r - SPLIT
            src[cursor : cursor + cntk] = r.astype(np.int16)
            colf[cursor : cursor + cntk] = core["c"][a:b].astype(np.float32)
            nrm[cursor : cursor + cntk] = core["n"][a:b]
            cursor += span
        col2d = np.ascontiguousarray(colf.reshape(S, P).T)
        nrm2d = np.ascontiguousarray(nrm.reshape(S, P).T)
        outs.append((src, col2d, nrm2d))
    return outs


def _wrap_idx(src_flat, S):
    """dma_gather index layout: per gather group of <=GCH chunks, indices
    wrapped i -> [i%16, i//16] in 16 partitions, replicated 8x down the
    partition dim; groups concatenated along the free dim."""
    blocks = []
    for g0 in range(0, S, GCH):
        gg = min(GCH, S - g0)
        idx = src_flat[g0 * P : (g0 + gg) * P]
        arr = np.zeros((16, gg * 8), np.int16)
        n = len(idx)
        arr[np.arange(n) % 16, np.arange(n) // 16] = idx
        blocks.append(arr)
    return np.tile(np.concatenate(blocks, axis=1), (8, 1))


def _pad_rows(x, total_rows, total_cols):
    out = np.zeros((total_rows, total_cols), np.float32)
    x = np.asarray(x, np.float32)
    out[: x.shape[0], : x.shape[1]] = x
    return out


# ---------------------------------------------------------------- program
def _build_program(nA, nB):
    SA, SB = int(nA.sum()), int(nB.sum())
    nc = bacc.Bacc(None, num_devices=NCORES)
    f32 = mybir.dt.float32

    x_lo = nc.dram_tensor("x_lo", [SPLIT, F], f32, kind="ExternalInput")
    x_hi = nc.dram_tensor("x_hi", [N - SPLIT, F], f32, kind="ExternalInput")
    idxA_d = nc.dram_tensor("idxA", [P, 8 * SA], mybir.dt.int16, kind="ExternalInput")
    idxB_d = nc.dram_tensor("idxB", [P, 8 * SB], mybir.dt.int16, kind="ExternalInput")
    colA_d = nc.dram_tensor("colA", [P, SA], f32, kind="ExternalInput")
    nrmA_d = nc.dram_tensor("nrmA", [P, SA], f32, kind="ExternalInput")
    colB_d = nc.dram_tensor("colB", [P, SB], f32, kind="ExternalInput")
    nrmB_d = nc.dram_tensor("nrmB", [P, SB], f32, kind="ExternalInput")
    Wa_d = nc.dram_tensor("Wa", [F, F], f32, kind="ExternalInput")
    Wb_d = nc.dram_tensor("Wb", [F, F], f32, kind="ExternalInput")
    ba_d = nc.dram_tensor("ba", [1, F], f32, kind="ExternalInput")
    bb_d = nc.dram_tensor("bb", [1, F], f32, kind="ExternalInput")
    wfc_d = nc.dram_tensor("wfc", [F, 1], f32, kind="ExternalInput")
    bfc_d = nc.dram_tensor("bfc", [1, 1], f32, kind="ExternalInput")
    iota_d = nc.dram_tensor("iota", [P, P], f32, kind="ExternalInput")
    out_d = nc.dram_tensor("outv", [1, DPC // 2], f32, kind="ExternalOutput")

    ngroupsA = (SA + GCH - 1) // GCH
    ngroupsB = (SB + GCH - 1) // GCH

    with tile.TileContext(nc) as tc:
        with (
            tc.tile_pool(name="const", bufs=1) as cpool,
            tc.tile_pool(name="gbuf", bufs=3) as gpool,
            tc.tile_pool(name="lhsT", bufs=4) as lpool,
            tc.tile_pool(name="u", bufs=3) as upool,
            tc.tile_pool(name="o", bufs=3) as opool,
            tc.tile_pool(name="praw", bufs=2, space="PSUM") as praw,
            tc.tile_pool(name="pout", bufs=2, space="PSUM") as pout,
            tc.tile_pool(name="pfc", bufs=2, space="PSUM") as pfc,
            tc.tile_pool(name="dram", bufs=1, space="DRAM") as dpool,
        ):
            idxA = cpool.tile([P, 8 * SA], mybir.dt.int16)
            idxB = cpool.tile([P, 8 * SB], mybir.dt.int16)
            colA = cpool.tile([P, SA], f32)
            nrmA = cpool.tile([P, SA], f32)
            colB = cpool.tile([P, SB], f32)
            nrmB = cpool.tile([P, SB], f32)
            Wa = cpool.tile([F, F], f32)
            Wb = cpool.tile([F, F], f32)
            ba = cpool.tile([1, F], f32)
            bb = cpool.tile([1, F], f32)
            wfc = cpool.tile([F, 1], f32)
            bfc = cpool.tile([1, 1], f32)
            iota = cpool.tile([P, P], f32)
            ones_row = cpool.tile([1, P], f32)
            fc_row = cpool.tile([1, DPC // 2], f32)

            for sb, dr in (
                (idxA, idxA_d), (idxB, idxB_d), (colA, colA_d), (nrmA, nrmA_d),
                (colB, colB_d), (nrmB, nrmB_d), (Wa, Wa_d), (Wb, Wb_d),
                (ba, ba_d), (bb, bb_d), (wfc, wfc_d), (bfc, bfc_d), (iota, iota_d),
            ):
                nc.sync.dma_start(sb[:], dr[:])
            nc.vector.memset(ones_row[:], 1.0)

            o1loc = dpool.tile([DPC, F], f32)
            o1full = dpool.tile([N, F], f32)

            def conv(lo_ap, hi_ap, W, bias, is_conv2):
                # stream state: (idx sbuf, col, nrm, ngroups, S, gather tiles)
                st = {
                    0: dict(idx=idxA, col=colA, nrm=nrmA, S=SA, emitted=0, tiles={}),
                    1: dict(idx=idxB, col=colB, nrm=nrmB, S=SB, emitted=0, tiles={}),
                }
                src_ap = {0: lo_ap, 1: hi_ap}
                cursor = [0, 0]

                def ensure_group(stream, g):
                    s = st[stream]
                    while s["emitted"] <= g:
                        ge = s["emitted"]
                        gg = min(GCH, s["S"] - ge * GCH)
                        gt = gpool.tile([P, GCH, P], f32, tag="gbuf", name=f"gb{is_conv2}_{stream}_{ge}")
                        nc.gpsimd.dma_gather(
                            gt[:, :gg, :],
                            src_ap[stream],
                            s["idx"][:, ge * GCH * 8 : ge * GCH * 8 + gg * 8],
                            gg * P,
                            gg * P,
                            F,
                            single_packet=False,
                        )
                        s["tiles"][ge] = gt
                        # drop stale group refs so the pool can recycle slots
                        if ge - 3 in s["tiles"]:
                            del s["tiles"][ge - 3]
                        s["emitted"] += 1

                for t in range(NT):
                    dv = P if t < NT - 1 else LAST_DV
                    nchunks = int(nA[t] + nB[t])
                    acc = praw.tile([P, P], f32, tag="praw", name=f"acc{is_conv2}_{t}")
                    done = 0
                    for stream in (0, 1):
                        nprog = nA if stream == 0 else nB
                        s = st[stream]
                        for k in range(int(nprog[t])):
                            c = cursor[stream] + k
                            g, slot = divmod(c, GCH)
                            ensure_group(stream, g)
                            oh = lpool.tile([P, P], f32, tag="lhsT", name=f"oh{is_conv2}_{t}_{stream}_{k}")
                            nc.vector.tensor_scalar(
                                out=oh[:],
                                in0=iota[:],
                                scalar1=s["col"][:, c : c + 1],
                                scalar2=s["nrm"][:, c : c + 1],
                                op0=mybir.AluOpType.is_equal,
                                op1=mybir.AluOpType.mult,
                            )
                            nc.tensor.matmul(
                                acc[:],
                                s["tiles"][g][:, slot, :],
                                oh[:],
                                start=(done == 0),
                                stop=(done == nchunks - 1),
                            )
                            done += 1
                        cursor[stream] += int(nprog[t])

                    u = upool.tile([P, P], f32, tag="u", name=f"u{is_conv2}_{t}")
                    nc.vector.tensor_copy(u[:], acc[:])
                    vout = pout.tile([P, P], f32, tag="pout", name=f"v{is_conv2}_{t}")
                    if not is_conv2:
                        # node-major out1[d, fo] = u.T @ Wa + 1 (x) ba
                        nc.tensor.matmul(vout[:dv, :], u[:, :dv], W[:], start=True, stop=False)
                        nc.tensor.matmul(vout[:dv, :], ones_row[:, :dv], bias[:], start=False, stop=True)
                        o = opool.tile([P, F], f32, tag="o", name=f"o1_{t}")
                        nc.vector.tensor_copy(o[:dv, :], vout[:dv, :])
                        nc.sync.dma_start(o1loc[:][t * P : t * P + dv, :], o[:dv, :])
                    else:
                        # feat-major v2[fo, d] = Wb.T @ u + bb (x) 1
                        nc.tensor.matmul(vout[:, :dv], W[:], u[:, :dv], start=True, stop=False)
                        nc.tensor.matmul(vout[:, :dv], bias[:], ones_row[:, :dv], start=False, stop=True)
                        sbuf = opool.tile([P, P], f32, tag="o", name=f"s2_{t}")
                        nc.vector.tensor_copy(sbuf[:, :dv], vout[:, :dv])
                        cv = dv // 2
                        pooled = opool.tile([P, P // 2], f32, tag="pool", name=f"pl_{t}")
                        nc.vector.tensor_tensor(
                            out=pooled[:, :cv],
                            in0=sbuf[:, 0:dv:2],
                            in1=sbuf[:, 1:dv:2],
                            op=mybir.AluOpType.max,
                        )
                        fcp = pfc.tile([1, P // 2], f32, tag="pfc", name=f"fc_{t}")
                        nc.tensor.matmul(fcp[:1, :cv], wfc[:], pooled[:, :cv], start=True, stop=True)
                        nc.vector.tensor_copy(fc_row[:, t * (P // 2) : t * (P // 2) + cv], fcp[:1, :cv])

            conv(x_lo[:], x_hi[:], Wa, ba, False)
            nc.gpsimd.collective_compute(
                "AllGather",
                mybir.AluOpType.bypass,
                replica_groups=[[0, 1, 2, 3], [4, 5, 6, 7]],
                ins=[o1loc.opt()],
                outs=[o1full.opt()],
            )
            full = o1full[:]
            conv(full[0:SPLIT, :], full[SPLIT:N, :], Wb, bb, True)

            sig = opool.tile([1, DPC // 2], f32, tag="sig")
            nc.scalar.activation(
                sig[:], fc_row[:], mybir.ActivationFunctionType.Sigmoid,
                bias=bfc[:1, :1], scale=1.0,
            )
            nc.sync.dma_start(out_d[:], sig[:])

    nc.compile()
    return nc


# ---------------------------------------------------------------- driver
def kernel(**inputs):
    global LAST_EXEC_NS
    import os

    x1 = np.asarray(inputs["x1"], np.float32)
    x2 = np.asarray(inputs["x2"], np.float32)
    towers = [
        dict(
            x=x1,
            cores=_edge_streams(x1, np.asarray(inputs["edge_index1"]), inputs["edge_weight1"]),
            Wa=np.asarray(inputs["W1a"], np.float32),
            Wb=np.asarray(inputs["W1b"], np.float32),
            ba=np.asarray(inputs["b1a"], np.float32),
            bb=np.asarray(inputs["b1b"], np.float32),
            fin=x1.shape[1],
        ),
        dict(
            x=x2,
            cores=_edge_streams(x2, np.asarray(inputs["edge_index2"]), inputs["edge_weight2"]),
            Wa=np.asarray(inputs["W2a"], np.float32),
            Wb=np.asarray(inputs["W2b"], np.float32),
            ba=np.asarray(inputs["b2a"], np.float32),
            bb=np.asarray(inputs["b2b"], np.float32),
            fin=x2.shape[1],
        ),
    ]

    # program-uniform chunk counts: max over all 8 cores
    nA = np.zeros(NT, np.int64)
    nB = np.zeros(NT, np.int64)
    for tw in towers:
        for core in tw["cores"]:
            cnt = core["cnt"].reshape(NT, 2)
            nA = np.maximum(nA, (cnt[:, 0] + P - 1) // P)
            nB = np.maximum(nB, (cnt[:, 1] + P - 1) // P)
    SA, SB = int(nA.sum()), int(nB.sum())

    iota = np.broadcast_to(np.arange(P, dtype=np.float32), (P, P)).copy()
    wfc = np.asarray(inputs["Wfc"], np.float32).reshape(F, 1)
    bfc = np.asarray(inputs["bfc"], np.float32).reshape(1, 1)

    in_maps = []
    for cid in range(NCORES):
        tw = towers[cid // CPT]
        core = tw["cores"][cid % CPT]
        (srcA, colA, nrmA), (srcB, colB, nrmB) = _pack_core(core, nA, nB)
        fin = tw["fin"]
        Wa = np.zeros((F, F), np.float32)
        Wa[:fin, :] = tw["Wa"]
        xpad = _pad_rows(tw["x"], N, F)
        in_maps.append(
            {
                "x_lo": xpad[:SPLIT],
                "x_hi": xpad[SPLIT:],
                "idxA": _wrap_idx(srcA, SA),
                "idxB": _wrap_idx(srcB, SB),
                "colA": colA,
                "nrmA": nrmA,
                "colB": colB,
                "nrmB": nrmB,
                "Wa": Wa,
                "Wb": tw["Wb"].astype(np.float32),
                "ba": tw["ba"].reshape(1, F).astype(np.float32),
                "bb": tw["bb"].reshape(1, F).astype(np.float32),
                "wfc": wfc,
                "bfc": bfc,
                "iota": iota,
            }
        )

    nc = _build_program(nA, nB)

    trace = bool(int(os.environ.get("KERNEL_TRACE", "0")))
    if trace:
        _install_trace_shim()
    res = run_bass_kernel_spmd(nc, in_maps, list(range(NCORES)), trace=trace)
    LAST_EXEC_NS = res.exec_time_ns

    parts = [res.results[cid]["outv"].reshape(-1) for cid in range(NCORES)]
    return np.concatenate(parts).reshape(N, 1).astype(np.float32)


def _install_trace_shim():
    """Provide antenv.axon_hooks (absent in this image) so
    run_bass_kernel_spmd(trace=True) can drive NTFF profiling, and stub the
    artifact upload."""
    import contextlib
    import ctypes
    import types

    import concourse.bass_utils as bu

    bu.upload_artifacts = lambda tmpdir: ""

    so_path = "/opt/axon/libaxon_pjrt.so"
    lib = ctypes.CDLL(so_path)
    if not hasattr(lib, "axon_start_nrt_profile"):
        return
    lib.axon_start_nrt_profile.argtypes = [ctypes.POINTER(ctypes.c_int64), ctypes.c_size_t]
    lib.axon_start_nrt_profile.restype = ctypes.c_int64
    lib.axon_stop_nrt_profile.argtypes = [ctypes.c_char_p]
    lib.axon_stop_nrt_profile.restype = ctypes.c_int64

    @contextlib.contextmanager
    def _hook(output_dir, device_ids):
        import jax

        jax.devices()
        if device_ids:
            ids = (ctypes.c_int64 * len(device_ids))(*device_ids)
            rc = lib.axon_start_nrt_profile(ids, len(device_ids))
        else:
            rc = lib.axon_start_nrt_profile(None, 0)
        if rc != 0:
            raise RuntimeError(f"axon_start_nrt_profile rc={rc}")
        try:
            yield
        finally:
            n = lib.axon_stop_nrt_profile(str(output_dir).encode())
            print(f"ntff profile: {n} file(s) -> {output_dir}")

    mod = types.ModuleType("antenv.axon_hooks")
    mod.get_axon_ntff_profile_hook = lambda: _hook
    mod.set_axon_ntff_profile_hook = lambda h: None
    sys.modules["antenv.axon_hooks"] = mod


# revision 6
# speedup vs baseline: 1.1614x; 1.1614x over previous
"""DualTowerGCN Trainium2 kernel.

Strategy (8 NeuronCores):
  - Model-parallel across towers: cores 0-3 run tower1, cores 4-7 run tower2
    (tower2's 64-dim input is zero-padded to 128 so one SPMD program serves
    both groups).
  - Within a tower, destinations are partitioned 4 ways (12500 nodes/core).
    Each GCN conv is a gather + one-hot-matmul scatter-add:
      * edges (incl. self loops, with symmetric-norm coefficients folded in)
        are sorted by destination tile (128 dests) and split by source half
        (dma_gather indices are int16, so the 50000-row table is addressed as
        two <=32768-row views);
      * dma_gather pulls 512B source rows in 4096-row batches;
      * per 128-edge chunk, a single dual-op DVE tensor_scalar builds
        onehot[e,d] = (iota==col[e]) * norm[e]; PE accumulates
        psum[f,d] += gathered[e,f].T @ onehot[e,d] over the tile's chunks;
      * the weight matrix is applied after the scatter (matmul associativity),
        bias via a rank-1 matmul accumulate.
  - One AllGather (groups [0-3], [4-7]) shares conv1 outputs for conv2.
  - Pairwise-max pooling (cluster = arange//2 in this workload's graclus
    stand-in) is a strided free-dim max; FC + sigmoid finish on device.
"""

import sys

sys.path.insert(0, "/opt/trn_rl_repo")

import numpy as np

import concourse.bacc as bacc
import concourse.tile as tile
from concourse import mybir
from concourse.bass_utils import run_bass_kernel_spmd

P = 128
N = 50000
E = 600000
F = 128  # feature width (tower2 padded 64 -> 128)
NCORES = 8
CPT = 4  # cores per tower
DPC = N // CPT  # dests per core = 12500
NT = (DPC + P - 1) // P  # dest tiles per core = 98
LAST_DV = DPC - (NT - 1) * P  # valid dests in last tile = 84
SPLIT = 32768  # int16 gather-index limit
GCH = 32  # chunks per dma_gather (4096 rows)

LAST_EXEC_NS = None


# ---------------------------------------------------------------- host prep
def _edge_streams(x_rows, ei, ew):
    """Per-tower edge preprocessing: add self loops, compute GCN symmetric
    norms, partition by dest core, sort by dest tile, split by source half.

    Returns per-core dicts of per-(tile, stream) edge lists."""
    row = np.asarray(ei[0], dtype=np.int64)
    col = np.asarray(ei[1], dtype=np.int64)
    w = np.asarray(ew, dtype=np.float32)

    deg = np.zeros(N, np.float32)
    np.add.at(deg, col, w)
    deg += 1.0  # self loops
    dinv = (1.0 / np.sqrt(deg)).astype(np.float32)

    rows = np.concatenate([row, np.arange(N, dtype=np.int64)])
    cols = np.concatenate([col, np.arange(N, dtype=np.int64)])
    norms = np.concatenate([dinv[row] * w * dinv[col], dinv * dinv]).astype(np.float32)

    cores = []
    for pcore in range(CPT):
        m = (cols // DPC) == pcore
        r, c, nr = rows[m], cols[m] - pcore * DPC, norms[m]
        t = c // P
        half = (r >= SPLIT).astype(np.int64)
        key = t * 2 + half
        order = np.argsort(key, kind="stable")
        r, c, nr, key = r[order], c[order], nr[order], key[order]
        cnt = np.bincount(key, minlength=NT * 2)
        off = np.concatenate([[0], np.cumsum(cnt)])
        cores.append({"r": r, "c": c % P, "n": nr, "cnt": cnt, "off": off})
    return cores


def _pack_core(core, nA, nB):
    """Lay one core's edges into the program-uniform chunk layout.

    Returns srcA/srcB (int16 flat, chunk-major), colA/colB + nrmA/nrmB
    ([128, S] f32)."""
    outs = []
    for half, nprog in ((0, nA), (1, nB)):
        S = int(nprog.sum())
        src = np.zeros(S * P, np.int16)
        colf = np.zeros(S * P, np.float32)
        nrm = np.zeros(S * P, np.float32)
        cursor = 0
        for t in range(NT):
            k = t * 2 + half
            a, b = core["off"][k], core["off"][k + 1]
            cntk = b - a
            span = int(nprog[t]) * P
            assert cntk <= span
            r = core["r"][a:b]
            if half:
                r = r - SPLIT
            src[cursor : cursor + cntk] = r.astype(np.int16)
            colf[cursor : cursor + cntk] = core["c"][a:b].astype(np.float32)
            nrm[cursor : cursor + cntk] = core["n"][a:b]
            cursor += span
        col2d = np.ascontiguousarray(colf.reshape(S, P).T)
        nrm2d = np.ascontiguousarray(nrm.reshape(S, P).T)
        outs.append((src, col2d, nrm2d))
    return outs


def _wrap_idx(src_flat, S):
    """dma_gather index layout: per gather group of <=GCH chunks, indices
    wrapped i -> [i%16, i//16] in 16 partitions, replicated 8x down the
    partition dim; groups concatenated along the free dim."""
    blocks = []
    for g0 in range(0, S, GCH):
        gg = min(GCH, S - g0)
        idx = src_flat[g0 * P : (g0 + gg) * P]
        arr = np.zeros((16, gg * 8), np.int16)
        n = len(idx)
        arr[np.arange(n) % 16, np.arange(n) // 16] = idx
        blocks.append(arr)
    return np.tile(np.concatenate(blocks, axis=1), (8, 1))


def _pad_rows(x, total_rows, total_cols):
    out = np.zeros((total_rows, total_cols), np.float32)
    x = np.asarray(x, np.float32)
    out[: x.shape[0], : x.shape[1]] = x
    return out


# ---------------------------------------------------------------- program
def _build_program(nA, nB):
    SA, SB = int(nA.sum()), int(nB.sum())
    nc = bacc.Bacc(None, num_devices=NCORES, num_swdge_queues=4)
    f32 = mybir.dt.float32

    x_lo = nc.dram_tensor("x_lo", [SPLIT, F], f32, kind="ExternalInput")
    x_hi = nc.dram_tensor("x_hi", [N - SPLIT, F], f32, kind="ExternalInput")
    idxA_d = nc.dram_tensor("idxA", [P, 8 * SA], mybir.dt.int16, kind="ExternalInput")
    idxB_d = nc.dram_tensor("idxB", [P, 8 * SB], mybir.dt.int16, kind="ExternalInput")
    colA_d = nc.dram_tensor("colA", [P, SA], f32, kind="ExternalInput")
    nrmA_d = nc.dram_tensor("nrmA", [P, SA], f32, kind="ExternalInput")
    colB_d = nc.dram_tensor("colB", [P, SB], f32, kind="ExternalInput")
    nrmB_d = nc.dram_tensor("nrmB", [P, SB], f32, kind="ExternalInput")
    Wa_d = nc.dram_tensor("Wa", [F, F], f32, kind="ExternalInput")
    Wb_d = nc.dram_tensor("Wb", [F, F], f32, kind="ExternalInput")
    ba_d = nc.dram_tensor("ba", [1, F], f32, kind="ExternalInput")
    bb_d = nc.dram_tensor("bb", [1, F], f32, kind="ExternalInput")
    wfc_d = nc.dram_tensor("wfc", [F, 1], f32, kind="ExternalInput")
    bfc_d = nc.dram_tensor("bfc", [1, 1], f32, kind="ExternalInput")
    iota_d = nc.dram_tensor("iota", [P, P], f32, kind="ExternalInput")
    out_d = nc.dram_tensor("outv", [1, DPC // 2], f32, kind="ExternalOutput")

    ngroupsA = (SA + GCH - 1) // GCH
    ngroupsB = (SB + GCH - 1) // GCH

    with tile.TileContext(nc) as tc:
        with (
            tc.tile_pool(name="const", bufs=1) as cpool,
            tc.tile_pool(name="gbuf", bufs=3) as gpool,
            tc.tile_pool(name="lhsT", bufs=4) as lpool,
            tc.tile_pool(name="u", bufs=3) as upool,
            tc.tile_pool(name="o", bufs=3) as opool,
            tc.tile_pool(name="praw", bufs=2, space="PSUM") as praw,
            tc.tile_pool(name="pout", bufs=2, space="PSUM") as pout,
            tc.tile_pool(name="pfc", bufs=2, space="PSUM") as pfc,
            tc.tile_pool(name="dram", bufs=1, space="DRAM") as dpool,
        ):
            idxA = cpool.tile([P, 8 * SA], mybir.dt.int16)
            idxB = cpool.tile([P, 8 * SB], mybir.dt.int16)
            colA = cpool.tile([P, SA], f32)
            nrmA = cpool.tile([P, SA], f32)
            colB = cpool.tile([P, SB], f32)
            nrmB = cpool.tile([P, SB], f32)
            Wa = cpool.tile([F, F], f32)
            Wb = cpool.tile([F, F], f32)
            ba = cpool.tile([1, F], f32)
            bb = cpool.tile([1, F], f32)
            wfc = cpool.tile([F, 1], f32)
            bfc = cpool.tile([1, 1], f32)
            iota = cpool.tile([P, P], f32)
            ones_row = cpool.tile([1, P], f32)
            fc_row = cpool.tile([1, DPC // 2], f32)

            for sb, dr in (
                (idxA, idxA_d), (idxB, idxB_d), (colA, colA_d), (nrmA, nrmA_d),
                (colB, colB_d), (nrmB, nrmB_d), (Wa, Wa_d), (Wb, Wb_d),
                (ba, ba_d), (bb, bb_d), (wfc, wfc_d), (bfc, bfc_d), (iota, iota_d),
            ):
                nc.sync.dma_start(sb[:], dr[:])
            nc.vector.memset(ones_row[:], 1.0)

            o1loc = dpool.tile([DPC, F], f32)
            o1full = dpool.tile([N, F], f32)

            def conv(lo_ap, hi_ap, W, bias, is_conv2):
                # stream state: (idx sbuf, col, nrm, ngroups, S, gather tiles)
                st = {
                    0: dict(idx=idxA, col=colA, nrm=nrmA, S=SA, emitted=0, tiles={}),
                    1: dict(idx=idxB, col=colB, nrm=nrmB, S=SB, emitted=0, tiles={}),
                }
                src_ap = {0: lo_ap, 1: hi_ap}
                cursor = [0, 0]

                def ensure_group(stream, g):
                    s = st[stream]
                    while s["emitted"] <= g:
                        ge = s["emitted"]
                        gg = min(GCH, s["S"] - ge * GCH)
                        gt = gpool.tile([P, GCH, P], f32, tag="gbuf", name=f"gb{is_conv2}_{stream}_{ge}")
                        nc.gpsimd.dma_gather(
                            gt[:, :gg, :],
                            src_ap[stream],
                            s["idx"][:, ge * GCH * 8 : ge * GCH * 8 + gg * 8],
                            gg * P,
                            gg * P,
                            F,
                            single_packet=False,
                            queue_num=(2 * stream + ge) % 4,
                        )
                        s["tiles"][ge] = gt
                        # drop stale group refs so the pool can recycle slots
                        if ge - 3 in s["tiles"]:
                            del s["tiles"][ge - 3]
                        s["emitted"] += 1

                for t in range(NT):
                    dv = P if t < NT - 1 else LAST_DV
                    nchunks = int(nA[t] + nB[t])
                    acc = praw.tile([P, P], f32, tag="praw", name=f"acc{is_conv2}_{t}")
                    done = 0
                    for stream in (0, 1):
                        nprog = nA if stream == 0 else nB
                        s = st[stream]
                        for k in range(int(nprog[t])):
                            c = cursor[stream] + k
                            g, slot = divmod(c, GCH)
                            ensure_group(stream, g)
                            oh = lpool.tile([P, P], f32, tag="lhsT", name=f"oh{is_conv2}_{t}_{stream}_{k}")
                            nc.vector.tensor_tensor(
                                out=oh[:],
                                in0=iota[:],
                                in1=s["col"][:, c : c + 1].to_broadcast([P, P]),
                                op=mybir.AluOpType.is_equal,
                            )
                            ohs = lpool.tile([P, P], f32, tag="ohs", name=f"os{is_conv2}_{t}_{stream}_{k}")
                            nc.scalar.activation(
                                ohs[:], oh[:], mybir.ActivationFunctionType.Copy,
                                scale=s["nrm"][:, c : c + 1],
                            )
                            nc.tensor.matmul(
                                acc[:],
                                s["tiles"][g][:, slot, :],
                                ohs[:],
                                start=(done == 0),
                                stop=(done == nchunks - 1),
                            )
                            done += 1
                        cursor[stream] += int(nprog[t])

                    u = upool.tile([P, P], f32, tag="u", name=f"u{is_conv2}_{t}")
                    nc.vector.tensor_copy(u[:], acc[:])
                    vout = pout.tile([P, P], f32, tag="pout", name=f"v{is_conv2}_{t}")
                    if not is_conv2:
                        # node-major out1[d, fo] = u.T @ Wa + 1 (x) ba
                        nc.tensor.matmul(vout[:dv, :], u[:, :dv], W[:], start=True, stop=False)
                        nc.tensor.matmul(vout[:dv, :], ones_row[:, :dv], bias[:], start=False, stop=True)
                        o = opool.tile([P, F], f32, tag="o", name=f"o1_{t}")
                        nc.vector.tensor_copy(o[:dv, :], vout[:dv, :])
                        nc.sync.dma_start(o1loc[:][t * P : t * P + dv, :], o[:dv, :])
                    else:
                        # feat-major v2[fo, d] = Wb.T @ u + bb (x) 1
                        nc.tensor.matmul(vout[:, :dv], W[:], u[:, :dv], start=True, stop=False)
                        nc.tensor.matmul(vout[:, :dv], bias[:], ones_row[:, :dv], start=False, stop=True)
                        sbuf = opool.tile([P, P], f32, tag="o", name=f"s2_{t}")
                        nc.vector.tensor_copy(sbuf[:, :dv], vout[:, :dv])
                        cv = dv // 2
                        pooled = opool.tile([P, P // 2], f32, tag="pool", name=f"pl_{t}")
                        nc.vector.tensor_tensor(
                            out=pooled[:, :cv],
                            in0=sbuf[:, 0:dv:2],
                            in1=sbuf[:, 1:dv:2],
                            op=mybir.AluOpType.max,
                        )
                        fcp = pfc.tile([1, P // 2], f32, tag="pfc", name=f"fc_{t}")
                        nc.tensor.matmul(fcp[:1, :cv], wfc[:], pooled[:, :cv], start=True, stop=True)
                        nc.vector.tensor_copy(fc_row[:, t * (P // 2) : t * (P // 2) + cv], fcp[:1, :cv])

            conv(x_lo[:], x_hi[:], Wa, ba, False)
            nc.gpsimd.collective_compute(
                "AllGather",
                mybir.AluOpType.bypass,
                replica_groups=[[0, 1, 2, 3], [4, 5, 6, 7]],
                ins=[o1loc.opt()],
                outs=[o1full.opt()],
            )
            full = o1full[:]
            conv(full[0:SPLIT, :], full[SPLIT:N, :], Wb, bb, True)

            sig = opool.tile([1, DPC // 2], f32, tag="sig")
            nc.scalar.activation(
                sig[:], fc_row[:], mybir.ActivationFunctionType.Sigmoid,
                bias=bfc[:1, :1], scale=1.0,
            )
            nc.sync.dma_start(out_d[:], sig[:])

    nc.compile()
    return nc


# ---------------------------------------------------------------- driver
def kernel(**inputs):
    global LAST_EXEC_NS
    import os

    x1 = np.asarray(inputs["x1"], np.float32)
    x2 = np.asarray(inputs["x2"], np.float32)
    towers = [
        dict(
            x=x1,
            cores=_edge_streams(x1, np.asarray(inputs["edge_index1"]), inputs["edge_weight1"]),
            Wa=np.asarray(inputs["W1a"], np.float32),
            Wb=np.asarray(inputs["W1b"], np.float32),
            ba=np.asarray(inputs["b1a"], np.float32),
            bb=np.asarray(inputs["b1b"], np.float32),
            fin=x1.shape[1],
        ),
        dict(
            x=x2,
            cores=_edge_streams(x2, np.asarray(inputs["edge_index2"]), inputs["edge_weight2"]),
            Wa=np.asarray(inputs["W2a"], np.float32),
            Wb=np.asarray(inputs["W2b"], np.float32),
            ba=np.asarray(inputs["b2a"], np.float32),
            bb=np.asarray(inputs["b2b"], np.float32),
            fin=x2.shape[1],
        ),
    ]

    # program-uniform chunk counts: max over all 8 cores
    nA = np.zeros(NT, np.int64)
    nB = np.zeros(NT, np.int64)
    for tw in towers:
        for core in tw["cores"]:
            cnt = core["cnt"].reshape(NT, 2)
            nA = np.maximum(nA, (cnt[:, 0] + P - 1) // P)
            nB = np.maximum(nB, (cnt[:, 1] + P - 1) // P)
    SA, SB = int(nA.sum()), int(nB.sum())

    iota = np.broadcast_to(np.arange(P, dtype=np.float32), (P, P)).copy()
    wfc = np.asarray(inputs["Wfc"], np.float32).reshape(F, 1)
    bfc = np.asarray(inputs["bfc"], np.float32).reshape(1, 1)

    in_maps = []
    for cid in range(NCORES):
        tw = towers[cid // CPT]
        core = tw["cores"][cid % CPT]
        (srcA, colA, nrmA), (srcB, colB, nrmB) = _pack_core(core, nA, nB)
        fin = tw["fin"]
        Wa = np.zeros((F, F), np.float32)
        Wa[:fin, :] = tw["Wa"]
        xpad = _pad_rows(tw["x"], N, F)
        in_maps.append(
            {
                "x_lo": xpad[:SPLIT],
                "x_hi": xpad[SPLIT:],
                "idxA": _wrap_idx(srcA, SA),
                "idxB": _wrap_idx(srcB, SB),
                "colA": colA,
                "nrmA": nrmA,
                "colB": colB,
                "nrmB": nrmB,
                "Wa": Wa,
                "Wb": tw["Wb"].astype(np.float32),
                "ba": tw["ba"].reshape(1, F).astype(np.float32),
                "bb": tw["bb"].reshape(1, F).astype(np.float32),
                "wfc": wfc,
                "bfc": bfc,
                "iota": iota,
            }
        )

    nc = _build_program(nA, nB)

    trace = bool(int(os.environ.get("KERNEL_TRACE", "0")))
    if trace:
        _install_trace_shim()
    res = run_bass_kernel_spmd(nc, in_maps, list(range(NCORES)), trace=trace)
    LAST_EXEC_NS = res.exec_time_ns

    parts = [res.results[cid]["outv"].reshape(-1) for cid in range(NCORES)]
    return np.concatenate(parts).reshape(N, 1).astype(np.float32)


def _install_trace_shim():
    """Provide antenv.axon_hooks (absent in this image) so
    run_bass_kernel_spmd(trace=True) can drive NTFF profiling, and stub the
    artifact upload."""
    import contextlib
    import ctypes
    import types

    import concourse.bass_utils as bu

    bu.upload_artifacts = lambda tmpdir: ""

    so_path = "/opt/axon/libaxon_pjrt.so"
    lib = ctypes.CDLL(so_path)
    if not hasattr(lib, "axon_start_nrt_profile"):
        return
    lib.axon_start_nrt_profile.argtypes = [ctypes.POINTER(ctypes.c_int64), ctypes.c_size_t]
    lib.axon_start_nrt_profile.restype = ctypes.c_int64
    lib.axon_stop_nrt_profile.argtypes = [ctypes.c_char_p]
    lib.axon_stop_nrt_profile.restype = ctypes.c_int64

    @contextlib.contextmanager
    def _hook(output_dir, device_ids):
        import jax

        jax.devices()
        if device_ids:
            ids = (ctypes.c_int64 * len(device_ids))(*device_ids)
            rc = lib.axon_start_nrt_profile(ids, len(device_ids))
        else:
            rc = lib.axon_start_nrt_profile(None, 0)
        if rc != 0:
            raise RuntimeError(f"axon_start_nrt_profile rc={rc}")
        try:
            yield
        finally:
            n = lib.axon_stop_nrt_profile(str(output_dir).encode())
            print(f"ntff profile: {n} file(s) -> {output_dir}")

    mod = types.ModuleType("antenv.axon_hooks")
    mod.get_axon_ntff_profile_hook = lambda: _hook
    mod.set_axon_ntff_profile_hook = lambda h: None
    sys.modules["antenv.axon_hooks"] = mod


# revision 8
# speedup vs baseline: 1.5448x; 1.3301x over previous
"""DualTowerGCN Trainium2 kernel.

Strategy (8 NeuronCores):
  - Model-parallel across towers: cores 0-3 run tower1, cores 4-7 run tower2
    (tower2's 64-dim input is zero-padded to 128 so one SPMD program serves
    both groups).
  - Within a tower, destinations are partitioned 4 ways (12500 nodes/core).
    Each GCN conv is a gather + one-hot-matmul scatter-add:
      * edges (incl. self loops, with symmetric-norm coefficients folded in)
        are sorted by destination tile (128 dests) and split by source half
        (dma_gather indices are int16, so the 50000-row table is addressed as
        two <=32768-row views);
      * dma_gather pulls 512B source rows in 4096-row batches;
      * per 128-edge chunk, a single dual-op DVE tensor_scalar builds
        onehot[e,d] = (iota==col[e]) * norm[e]; PE accumulates
        psum[f,d] += gathered[e,f].T @ onehot[e,d] over the tile's chunks;
      * the weight matrix is applied after the scatter (matmul associativity),
        bias via a rank-1 matmul accumulate.
  - One AllGather (groups [0-3], [4-7]) shares conv1 outputs for conv2.
  - Pairwise-max pooling (cluster = arange//2 in this workload's graclus
    stand-in) is a strided free-dim max; FC + sigmoid finish on device.
"""

import sys

sys.path.insert(0, "/opt/trn_rl_repo")

import numpy as np

import concourse.bacc as bacc
import concourse.tile as tile
from concourse import mybir
from concourse.bass_utils import run_bass_kernel_spmd

P = 128
N = 50000
E = 600000
F = 128  # feature width (tower2 padded 64 -> 128)
NCORES = 8
CPT = 4  # cores per tower
DPC = N // CPT  # dests per core = 12500
NT = (DPC + P - 1) // P  # dest tiles per core = 98
LAST_DV = DPC - (NT - 1) * P  # valid dests in last tile = 84
SPLIT = 32768  # int16 gather-index limit
GCH = 16  # chunks per dma_gather (2048 rows)

LAST_EXEC_NS = None


# ---------------------------------------------------------------- host prep
def _edge_streams(x_rows, ei, ew):
    """Per-tower edge preprocessing: add self loops, compute GCN symmetric
    norms, partition by dest core, sort by dest tile, split by source half.

    Returns per-core dicts of per-(tile, stream) edge lists."""
    row = np.asarray(ei[0], dtype=np.int64)
    col = np.asarray(ei[1], dtype=np.int64)
    w = np.asarray(ew, dtype=np.float32)

    deg = np.zeros(N, np.float32)
    np.add.at(deg, col, w)
    deg += 1.0  # self loops
    dinv = (1.0 / np.sqrt(deg)).astype(np.float32)

    rows = np.concatenate([row, np.arange(N, dtype=np.int64)])
    cols = np.concatenate([col, np.arange(N, dtype=np.int64)])
    norms = np.concatenate([dinv[row] * w * dinv[col], dinv * dinv]).astype(np.float32)

    cores = []
    for pcore in range(CPT):
        m = (cols // DPC) == pcore
        r, c, nr = rows[m], cols[m] - pcore * DPC, norms[m]
        t = c // P
        half = (r >= SPLIT).astype(np.int64)
        key = t * 2 + half
        order = np.argsort(key, kind="stable")
        r, c, nr, key = r[order], c[order], nr[order], key[order]
        cnt = np.bincount(key, minlength=NT * 2)
        off = np.concatenate([[0], np.cumsum(cnt)])
        cores.append({"r": r, "c": c % P, "n": nr, "cnt": cnt, "off": off})
    return cores


def _pack_core(core, nA, nB):
    """Lay one core's edges into the program-uniform chunk layout.

    Returns srcA/srcB (int16 flat, chunk-major), colA/colB + nrmA/nrmB
    ([128, S] f32)."""
    outs = []
    for half, nprog in ((0, nA), (1, nB)):
        S = int(nprog.sum())
        src = np.zeros(S * P, np.int16)
        colf = np.zeros(S * P, np.float32)
        nrm = np.zeros(S * P, np.float32)
        cursor = 0
        for t in range(NT):
            k = t * 2 + half
            a, b = core["off"][k], core["off"][k + 1]
            cntk = b - a
            span = int(nprog[t]) * P
            assert cntk <= span
            r = core["r"][a:b]
            if half:
                r = r - SPLIT
            src[cursor : cursor + cntk] = r.astype(np.int16)
            colf[cursor : cursor + cntk] = core["c"][a:b].astype(np.float32)
            nrm[cursor : cursor + cntk] = core["n"][a:b]
            cursor += span
        col2d = np.ascontiguousarray(colf.reshape(S, P).T)
        nrm2d = np.ascontiguousarray(nrm.reshape(S, P).T)
        outs.append((src, col2d, nrm2d))
    return outs


def _wrap_idx(src_flat, S):
    """dma_gather index layout: per gather group of <=GCH chunks, indices
    wrapped i -> [i%16, i//16] in 16 partitions, replicated 8x down the
    partition dim; groups concatenated along the free dim."""
    blocks = []
    for g0 in range(0, S, GCH):
        gg = min(GCH, S - g0)
        idx = src_flat[g0 * P : (g0 + gg) * P]
        arr = np.zeros((16, gg * 8), np.int16)
        n = len(idx)
        arr[np.arange(n) % 16, np.arange(n) // 16] = idx
        blocks.append(arr)
    return np.tile(np.concatenate(blocks, axis=1), (8, 1))


def _pad_rows(x, total_rows, total_cols):
    out = np.zeros((total_rows, total_cols), np.float32)
    x = np.asarray(x, np.float32)
    out[: x.shape[0], : x.shape[1]] = x
    return out


# ---------------------------------------------------------------- program
def _build_program(nA, nB):
    SA, SB = int(nA.sum()), int(nB.sum())
    nc = bacc.Bacc(None, num_devices=NCORES, num_swdge_queues=4, dynamic_dma_scratch_size=49152)
    f32 = mybir.dt.float32

    x_lo = nc.dram_tensor("x_lo", [SPLIT, F], f32, kind="ExternalInput")
    x_hi = nc.dram_tensor("x_hi", [N - SPLIT, F], f32, kind="ExternalInput")
    idxA_d = nc.dram_tensor("idxA", [P, 8 * SA], mybir.dt.int16, kind="ExternalInput")
    idxB_d = nc.dram_tensor("idxB", [P, 8 * SB], mybir.dt.int16, kind="ExternalInput")
    colA_d = nc.dram_tensor("colA", [P, SA], f32, kind="ExternalInput")
    nrmA_d = nc.dram_tensor("nrmA", [P, SA], f32, kind="ExternalInput")
    colB_d = nc.dram_tensor("colB", [P, SB], f32, kind="ExternalInput")
    nrmB_d = nc.dram_tensor("nrmB", [P, SB], f32, kind="ExternalInput")
    Wa_d = nc.dram_tensor("Wa", [F, F], f32, kind="ExternalInput")
    Wb_d = nc.dram_tensor("Wb", [F, F], f32, kind="ExternalInput")
    ba_d = nc.dram_tensor("ba", [1, F], f32, kind="ExternalInput")
    bb_d = nc.dram_tensor("bb", [1, F], f32, kind="ExternalInput")
    wfc_d = nc.dram_tensor("wfc", [F, 1], f32, kind="ExternalInput")
    bfc_d = nc.dram_tensor("bfc", [1, 1], f32, kind="ExternalInput")
    iota_d = nc.dram_tensor("iota", [P, P], f32, kind="ExternalInput")
    out_d = nc.dram_tensor("outv", [1, DPC // 2], f32, kind="ExternalOutput")

    ngroupsA = (SA + GCH - 1) // GCH
    ngroupsB = (SB + GCH - 1) // GCH

    with tile.TileContext(nc) as tc:
        with (
            tc.tile_pool(name="const", bufs=1) as cpool,
            tc.tile_pool(name="gbuf", bufs=5) as gpool,
            tc.tile_pool(name="lhsT", bufs=4) as lpool,
            tc.tile_pool(name="u", bufs=3) as upool,
            tc.tile_pool(name="o", bufs=3) as opool,
            tc.tile_pool(name="praw", bufs=2, space="PSUM") as praw,
            tc.tile_pool(name="pout", bufs=2, space="PSUM") as pout,
            tc.tile_pool(name="pfc", bufs=2, space="PSUM") as pfc,
            tc.tile_pool(name="dram", bufs=1, space="DRAM") as dpool,
        ):
            idxA = cpool.tile([P, 8 * SA], mybir.dt.int16)
            idxB = cpool.tile([P, 8 * SB], mybir.dt.int16)
            colA = cpool.tile([P, SA], f32)
            nrmA = cpool.tile([P, SA], f32)
            colB = cpool.tile([P, SB], f32)
            nrmB = cpool.tile([P, SB], f32)
            Wa = cpool.tile([F, F], f32)
            Wb = cpool.tile([F, F], f32)
            ba = cpool.tile([1, F], f32)
            bb = cpool.tile([1, F], f32)
            wfc = cpool.tile([F, 1], f32)
            bfc = cpool.tile([1, 1], f32)
            iota = cpool.tile([P, P], f32)
            ones_row = cpool.tile([1, P], f32)
            fc_row = cpool.tile([1, DPC // 2], f32)

            for sb, dr in (
                (idxA, idxA_d), (idxB, idxB_d), (colA, colA_d), (nrmA, nrmA_d),
                (colB, colB_d), (nrmB, nrmB_d), (Wa, Wa_d), (Wb, Wb_d),
                (ba, ba_d), (bb, bb_d), (wfc, wfc_d), (bfc, bfc_d), (iota, iota_d),
            ):
                nc.sync.dma_start(sb[:], dr[:])
            nc.vector.memset(ones_row[:], 1.0)

            o1loc = dpool.tile([DPC, F], f32)
            o1full = dpool.tile([N, F], f32)

            def conv(lo_ap, hi_ap, W, bias, is_conv2):
                # stream state: (idx sbuf, col, nrm, ngroups, S, gather tiles)
                st = {
                    0: dict(idx=idxA, col=colA, nrm=nrmA, S=SA, emitted=0, tiles={}),
                    1: dict(idx=idxB, col=colB, nrm=nrmB, S=SB, emitted=0, tiles={}),
                }
                src_ap = {0: lo_ap, 1: hi_ap}
                cursor = [0, 0]

                def ensure_group(stream, g):
                    s = st[stream]
                    while s["emitted"] <= g:
                        ge = s["emitted"]
                        gg = min(GCH, s["S"] - ge * GCH)
                        gt = gpool.tile([P, GCH, P], f32, tag="gbuf", name=f"gb{is_conv2}_{stream}_{ge}")
                        nc.gpsimd.dma_gather(
                            gt[:, :gg, :],
                            src_ap[stream],
                            s["idx"][:, ge * GCH * 8 : ge * GCH * 8 + gg * 8],
                            gg * P,
                            gg * P,
                            F,
                            single_packet=False,
                            queue_num=(2 * stream + ge) % 4,
                        )
                        s["tiles"][ge] = gt
                        # drop stale group refs so the pool can recycle slots
                        if ge - 5 in s["tiles"]:
                            del s["tiles"][ge - 5]
                        s["emitted"] += 1

                for t in range(NT):
                    dv = P if t < NT - 1 else LAST_DV
                    nchunks = int(nA[t] + nB[t])
                    acc = praw.tile([P, P], f32, tag="praw", name=f"acc{is_conv2}_{t}")
                    done = 0
                    for stream in (0, 1):
                        nprog = nA if stream == 0 else nB
                        s = st[stream]
                        for k in range(int(nprog[t])):
                            c = cursor[stream] + k
                            g, slot = divmod(c, GCH)
                            ensure_group(stream, g)
                            oh = lpool.tile([P, P], f32, tag="lhsT", name=f"oh{is_conv2}_{t}_{stream}_{k}")
                            nc.vector.tensor_tensor(
                                out=oh[:],
                                in0=iota[:],
                                in1=s["col"][:, c : c + 1].to_broadcast([P, P]),
                                op=mybir.AluOpType.is_equal,
                            )
                            ohs = lpool.tile([P, P], f32, tag="ohs", name=f"os{is_conv2}_{t}_{stream}_{k}")
                            nc.scalar.activation(
                                ohs[:], oh[:], mybir.ActivationFunctionType.Copy,
                                scale=s["nrm"][:, c : c + 1],
                            )
                            nc.tensor.matmul(
                                acc[:],
                                s["tiles"][g][:, slot, :],
                                ohs[:],
                                start=(done == 0),
                                stop=(done == nchunks - 1),
                            )
                            done += 1
                        cursor[stream] += int(nprog[t])

                    u = upool.tile([P, P], f32, tag="u", name=f"u{is_conv2}_{t}")
                    nc.vector.tensor_copy(u[:], acc[:])
                    vout = pout.tile([P, P], f32, tag="pout", name=f"v{is_conv2}_{t}")
                    if not is_conv2:
                        # node-major out1[d, fo] = u.T @ Wa + 1 (x) ba
                        nc.tensor.matmul(vout[:dv, :], u[:, :dv], W[:], start=True, stop=False)
                        nc.tensor.matmul(vout[:dv, :], ones_row[:, :dv], bias[:], start=False, stop=True)
                        o = opool.tile([P, F], f32, tag="o", name=f"o1_{t}")
                        nc.vector.tensor_copy(o[:dv, :], vout[:dv, :])
                        nc.sync.dma_start(o1loc[:][t * P : t * P + dv, :], o[:dv, :])
                    else:
                        # feat-major v2[fo, d] = Wb.T @ u + bb (x) 1
                        nc.tensor.matmul(vout[:, :dv], W[:], u[:, :dv], start=True, stop=False)
                        nc.tensor.matmul(vout[:, :dv], bias[:], ones_row[:, :dv], start=False, stop=True)
                        sbuf = opool.tile([P, P], f32, tag="o", name=f"s2_{t}")
                        nc.vector.tensor_copy(sbuf[:, :dv], vout[:, :dv])
                        cv = dv // 2
                        pooled = opool.tile([P, P // 2], f32, tag="pool", name=f"pl_{t}")
                        nc.vector.tensor_tensor(
                            out=pooled[:, :cv],
                            in0=sbuf[:, 0:dv:2],
                            in1=sbuf[:, 1:dv:2],
                            op=mybir.AluOpType.max,
                        )
                        fcp = pfc.tile([1, P // 2], f32, tag="pfc", name=f"fc_{t}")
                        nc.tensor.matmul(fcp[:1, :cv], wfc[:], pooled[:, :cv], start=True, stop=True)
                        nc.vector.tensor_copy(fc_row[:, t * (P // 2) : t * (P // 2) + cv], fcp[:1, :cv])

            conv(x_lo[:], x_hi[:], Wa, ba, False)
            nc.gpsimd.collective_compute(
                "AllGather",
                mybir.AluOpType.bypass,
                replica_groups=[[0, 1, 2, 3], [4, 5, 6, 7]],
                ins=[o1loc.opt()],
                outs=[o1full.opt()],
            )
            full = o1full[:]
            conv(full[0:SPLIT, :], full[SPLIT:N, :], Wb, bb, True)

            nc.scalar.activation(
                fc_row[:], fc_row[:], mybir.ActivationFunctionType.Sigmoid,
                bias=bfc[:1, :1], scale=1.0,
            )
            nc.sync.dma_start(out_d[:], fc_row[:])

    nc.compile()
    return nc


# ---------------------------------------------------------------- driver
def kernel(**inputs):
    global LAST_EXEC_NS
    import os

    x1 = np.asarray(inputs["x1"], np.float32)
    x2 = np.asarray(inputs["x2"], np.float32)
    towers = [
        dict(
            x=x1,
            cores=_edge_streams(x1, np.asarray(inputs["edge_index1"]), inputs["edge_weight1"]),
            Wa=np.asarray(inputs["W1a"], np.float32),
            Wb=np.asarray(inputs["W1b"], np.float32),
            ba=np.asarray(inputs["b1a"], np.float32),
            bb=np.asarray(inputs["b1b"], np.float32),
            fin=x1.shape[1],
        ),
        dict(
            x=x2,
            cores=_edge_streams(x2, np.asarray(inputs["edge_index2"]), inputs["edge_weight2"]),
            Wa=np.asarray(inputs["W2a"], np.float32),
            Wb=np.asarray(inputs["W2b"], np.float32),
            ba=np.asarray(inputs["b2a"], np.float32),
            bb=np.asarray(inputs["b2b"], np.float32),
            fin=x2.shape[1],
        ),
    ]

    # program-uniform chunk counts: max over all 8 cores
    nA = np.zeros(NT, np.int64)
    nB = np.zeros(NT, np.int64)
    for tw in towers:
        for core in tw["cores"]:
            cnt = core["cnt"].reshape(NT, 2)
            nA = np.maximum(nA, (cnt[:, 0] + P - 1) // P)
            nB = np.maximum(nB, (cnt[:, 1] + P - 1) // P)
    SA, SB = int(nA.sum()), int(nB.sum())

    iota = np.broadcast_to(np.arange(P, dtype=np.float32), (P, P)).copy()
    wfc = np.asarray(inputs["Wfc"], np.float32).reshape(F, 1)
    bfc = np.asarray(inputs["bfc"], np.float32).reshape(1, 1)

    in_maps = []
    for cid in range(NCORES):
        tw = towers[cid // CPT]
        core = tw["cores"][cid % CPT]
        (srcA, colA, nrmA), (srcB, colB, nrmB) = _pack_core(core, nA, nB)
        fin = tw["fin"]
        Wa = np.zeros((F, F), np.float32)
        Wa[:fin, :] = tw["Wa"]
        xpad = _pad_rows(tw["x"], N, F)
        in_maps.append(
            {
                "x_lo": xpad[:SPLIT],
                "x_hi": xpad[SPLIT:],
                "idxA": _wrap_idx(srcA, SA),
                "idxB": _wrap_idx(srcB, SB),
                "colA": colA,
                "nrmA": nrmA,
                "colB": colB,
                "nrmB": nrmB,
                "Wa": Wa,
                "Wb": tw["Wb"].astype(np.float32),
                "ba": tw["ba"].reshape(1, F).astype(np.float32),
                "bb": tw["bb"].reshape(1, F).astype(np.float32),
                "wfc": wfc,
                "bfc": bfc,
                "iota": iota,
            }
        )

    nc = _build_program(nA, nB)

    trace = bool(int(os.environ.get("KERNEL_TRACE", "0")))
    if trace:
        _install_trace_shim()
    res = run_bass_kernel_spmd(nc, in_maps, list(range(NCORES)), trace=trace)
    LAST_EXEC_NS = res.exec_time_ns

    parts = [res.results[cid]["outv"].reshape(-1) for cid in range(NCORES)]
    return np.concatenate(parts).reshape(N, 1).astype(np.float32)


def _install_trace_shim():
    """Provide antenv.axon_hooks (absent in this image) so
    run_bass_kernel_spmd(trace=True) can drive NTFF profiling, and stub the
    artifact upload."""
    import contextlib
    import ctypes
    import types

    import concourse.bass_utils as bu

    bu.upload_artifacts = lambda tmpdir: ""

    so_path = "/opt/axon/libaxon_pjrt.so"
    lib = ctypes.CDLL(so_path)
    if not hasattr(lib, "axon_start_nrt_profile"):
        return
    lib.axon_start_nrt_profile.argtypes = [ctypes.POINTER(ctypes.c_int64), ctypes.c_size_t]
    lib.axon_start_nrt_profile.restype = ctypes.c_int64
    lib.axon_stop_nrt_profile.argtypes = [ctypes.c_char_p]
    lib.axon_stop_nrt_profile.restype = ctypes.c_int64

    @contextlib.contextmanager
    def _hook(output_dir, device_ids):
        import jax

        jax.devices()
        if device_ids:
            ids = (ctypes.c_int64 * len(device_ids))(*device_ids)
            rc = lib.axon_start_nrt_profile(ids, len(device_ids))
        else:
            rc = lib.axon_start_nrt_profile(None, 0)
        if rc != 0:
            raise RuntimeError(f"axon_start_nrt_profile rc={rc}")
        try:
            yield
        finally:
            n = lib.axon_stop_nrt_profile(str(output_dir).encode())
            print(f"ntff profile: {n} file(s) -> {output_dir}")

    mod = types.ModuleType("antenv.axon_hooks")
    mod.get_axon_ntff_profile_hook = lambda: _hook
    mod.set_axon_ntff_profile_hook = lambda h: None
    sys.modules["antenv.axon_hooks"] = mod


# revision 9
# speedup vs baseline: 1.7646x; 1.1423x over previous
"""DualTowerGCN Trainium2 kernel.

Strategy (8 NeuronCores):
  - Model-parallel across towers: cores 0-3 run tower1, cores 4-7 run tower2
    (tower2's 64-dim input is zero-padded to 128 so one SPMD program serves
    both groups).
  - Within a tower, destinations are partitioned 4 ways (12500 nodes/core).
    Destination PAIRS (the arange//2 graclus clusters) are re-assigned to
    128-dest tiles with an LPT balance so per-tile edge counts are uniform
    across tiles and cores (minimizes chunk padding, SPMD program is uniform).
  - Each GCN conv is a gather + one-hot-matmul scatter-add:
      * edges (incl. self loops, with symmetric-norm coefficients folded in)
        are bucketed by destination tile and split by source half (dma_gather
        indices are int16, so tables are addressed as two <=32768-row views);
      * dma_gather pulls source rows in multi-chunk batches across 4 SWDGE
        queues with an enlarged descriptor-ring carveout;
      * per 128-edge chunk, DVE builds onehot[e,d] = (iota==col[e]) and the
        otherwise-idle ACT engine scales it by norm[e]; PE accumulates
        psum[f,d] += gathered[e,f].T @ onehot[e,d] over the tile's chunks;
      * the weight matrix is applied after the scatter (matmul associativity),
        bias via a rank-1 matmul accumulate.
  - The inter-conv AllGather (groups [0-3], [4-7]) is split into 4 chunks so
    it overlaps conv1 compute; conv2's gather indices are remapped host-side
    to the chunk-major allgather layout.
  - Pairwise-max pooling is a strided free-dim max; FC + sigmoid on device;
    the host inverts the pair permutation on the tiny [6250] outputs.

KERNEL_BF16=1 stores the gather tables (x and conv1 output) in bf16, halving
the dominant random-gather HBM traffic; PSUM accumulation and all
weight/bias/FC math stay fp32.
"""

import os
import sys

sys.path.insert(0, "/opt/trn_rl_repo")

import numpy as np

import concourse.bacc as bacc
import concourse.tile as tile
from concourse import mybir
from concourse.bass_utils import run_bass_kernel_spmd

P = 128
N = 50000
E = 600000
F = 128  # feature width (tower2 padded 64 -> 128)
NCORES = 8
CPT = 4  # cores per tower
DPC = N // CPT  # dests per core = 12500
NPAIR = DPC // 2  # cluster pairs per core = 6250
NT = (DPC + P - 1) // P  # dest tiles per core = 98
LAST_DV = DPC - (NT - 1) * P  # valid dests in last tile = 84
SPLIT = 32768  # int16 gather-index limit
GCH = 16  # chunks per dma_gather
AGCH = [0, 3200, 6400, 9600, 12500]  # allgather chunk row boundaries

BF16 = bool(int(os.environ.get("KERNEL_BF16", "0")))

LAST_EXEC_NS = None


# ---------------------------------------------------------------- host prep
def _edges_by_core(ei, ew):
    """Add self loops + GCN symmetric norms; partition by dest core.
    Returns per-core (src_global, dest_local, norm)."""
    row = np.asarray(ei[0], dtype=np.int64)
    col = np.asarray(ei[1], dtype=np.int64)
    w = np.asarray(ew, dtype=np.float32)

    deg = np.zeros(N, np.float32)
    np.add.at(deg, col, w)
    deg += 1.0
    dinv = (1.0 / np.sqrt(deg)).astype(np.float32)

    rows = np.concatenate([row, np.arange(N, dtype=np.int64)])
    cols = np.concatenate([col, np.arange(N, dtype=np.int64)])
    norms = np.concatenate([dinv[row] * w * dinv[col], dinv * dinv]).astype(np.float32)

    cores = []
    for pcore in range(CPT):
        m = (cols // DPC) == pcore
        cores.append((rows[m], cols[m] - pcore * DPC, norms[m]))
    return cores


def _balance_pairs(dest_local):
    """LPT-assign this core's 6250 pairs to 98 tiles (cap 64/../42) so
    per-tile edge counts are near-uniform. Returns (tile_of_pair,
    slot_of_pair)."""
    import heapq

    wt = np.bincount(dest_local // 2, minlength=NPAIR).astype(np.int64)
    order = np.argsort(-wt, kind="stable")
    caps = [64] * (NT - 1) + [LAST_DV // 2]
    heap = [(0.0, t) for t in range(NT)]
    heapq.heapify(heap)
    fill = [0] * NT
    tile_of = np.zeros(NPAIR, np.int32)
    slot_of = np.zeros(NPAIR, np.int32)
    for pr in order:
        while True:
            load, t = heapq.heappop(heap)
            if fill[t] < caps[t]:
                break
        tile_of[pr] = t
        slot_of[pr] = fill[t]
        fill[t] += 1
        if fill[t] < caps[t]:
            heapq.heappush(heap, (load + wt[pr] / (caps[t] / 64.0), t))
    return tile_of, slot_of


def _bucket_edges(src, dest_local, nrm, tile_of, slot_of, remap, split):
    """Map edges to (tile, half, tile-local col) under the pair permutation
    and source remap; sort by (tile, half)."""
    rm = remap[src] if remap is not None else src
    pair = dest_local // 2
    t = tile_of[pair]
    col_l = slot_of[pair] * 2 + (dest_local % 2)
    half = (rm >= split).astype(np.int64)
    key = t * 2 + half
    order = np.argsort(key, kind="stable")
    cnt = np.bincount(key[order], minlength=NT * 2)
    off = np.concatenate([[0], np.cumsum(cnt)])
    return {
        "r": rm[order],
        "c": col_l[order],
        "n": nrm[order],
        "cnt": cnt,
        "off": off,
    }


def _pack_core(core, nA, nB):
    """Lay one core's edges into the program-uniform chunk layout."""
    outs = []
    for half, nprog in ((0, nA), (1, nB)):
        S = int(nprog.sum())
        src = np.zeros(S * P, np.int32)
        colf = np.zeros(S * P, np.float32)
        nrm = np.zeros(S * P, np.float32)
        cursor = 0
        for t in range(NT):
            k = t * 2 + half
            a, b = core["off"][k], core["off"][k + 1]
            cntk = b - a
            span = int(nprog[t]) * P
            assert cntk <= span
            r = core["r"][a:b]
            if half:
                r = r - SPLIT
            src[cursor : cursor + cntk] = r
            colf[cursor : cursor + cntk] = core["c"][a:b].astype(np.float32)
            nrm[cursor : cursor + cntk] = core["n"][a:b]
            cursor += span
        assert src.max(initial=0) < SPLIT
        col2d = np.ascontiguousarray(colf.reshape(S, P).T)
        nrm2d = np.ascontiguousarray(nrm.reshape(S, P).T)
        outs.append((src.astype(np.int16), col2d, nrm2d))
    return outs


def _wrap_idx(src_flat, S):
    """dma_gather index layout: per gather group of <=GCH chunks, indices
    wrapped i -> [i%16, i//16] in 16 partitions, replicated 8x down the
    partition dim; groups concatenated along the free dim."""
    blocks = []
    for g0 in range(0, S, GCH):
        gg = min(GCH, S - g0)
        idx = src_flat[g0 * P : (g0 + gg) * P]
        arr = np.zeros((16, gg * 8), np.int16)
        n = len(idx)
        arr[np.arange(n) % 16, np.arange(n) // 16] = idx
        blocks.append(arr)
    return np.tile(np.concatenate(blocks, axis=1), (8, 1))


def _pos_map(tile_of, slot_of):
    """Local dest l -> row position in this core's (permuted) o1loc."""
    l = np.arange(DPC, dtype=np.int64)
    return tile_of[l // 2].astype(np.int64) * P + slot_of[l // 2] * 2 + (l % 2)


def _pad_rows(x, total_rows, total_cols):
    out = np.zeros((total_rows, total_cols), np.float32)
    x = np.asarray(x, np.float32)
    out[: x.shape[0], : x.shape[1]] = x
    return out


# ---------------------------------------------------------------- program
def _build_program(shape):
    nA1, nB1, nA2, nB2 = shape
    SA1, SB1 = int(nA1.sum()), int(nB1.sum())
    SA2, SB2 = int(nA2.sum()), int(nB2.sum())
    nc = bacc.Bacc(None, num_devices=NCORES, num_swdge_queues=4,
                   dynamic_dma_scratch_size=65536)
    f32 = mybir.dt.float32
    gdt = mybir.dt.bfloat16 if BF16 else f32

    x_lo = nc.dram_tensor("x_lo", [SPLIT, F], gdt, kind="ExternalInput")
    x_hi = nc.dram_tensor("x_hi", [N - SPLIT, F], gdt, kind="ExternalInput")
    decl = {}
    for cv, (sa, sb) in ((1, (SA1, SB1)), (2, (SA2, SB2))):
        decl[f"idxA{cv}"] = nc.dram_tensor(f"idxA{cv}", [P, 8 * sa], mybir.dt.int16, kind="ExternalInput")
        decl[f"idxB{cv}"] = nc.dram_tensor(f"idxB{cv}", [P, 8 * sb], mybir.dt.int16, kind="ExternalInput")
        decl[f"colA{cv}"] = nc.dram_tensor(f"colA{cv}", [P, sa], gdt, kind="ExternalInput")
        decl[f"nrmA{cv}"] = nc.dram_tensor(f"nrmA{cv}", [P, sa], f32, kind="ExternalInput")
        decl[f"colB{cv}"] = nc.dram_tensor(f"colB{cv}", [P, sb], gdt, kind="ExternalInput")
        decl[f"nrmB{cv}"] = nc.dram_tensor(f"nrmB{cv}", [P, sb], f32, kind="ExternalInput")
    Wa_d = nc.dram_tensor("Wa", [F, F], f32, kind="ExternalInput")
    Wb_d = nc.dram_tensor("Wb", [F, F], f32, kind="ExternalInput")
    ba_d = nc.dram_tensor("ba", [1, F], f32, kind="ExternalInput")
    bb_d = nc.dram_tensor("bb", [1, F], f32, kind="ExternalInput")
    wfc_d = nc.dram_tensor("wfc", [F, 1], f32, kind="ExternalInput")
    bfc_d = nc.dram_tensor("bfc", [1, 1], f32, kind="ExternalInput")
    iota_d = nc.dram_tensor("iota", [P, P], gdt, kind="ExternalInput")
    out_d = nc.dram_tensor("outv", [1, NPAIR], f32, kind="ExternalOutput")

    with tile.TileContext(nc) as tc:
        with (
            tc.tile_pool(name="const", bufs=1) as cpool,
            tc.tile_pool(name="gbuf", bufs=6) as gpool,
            tc.tile_pool(name="idxs", bufs=4) as ipool,
            tc.tile_pool(name="lhsT", bufs=4) as lpool,
            tc.tile_pool(name="u", bufs=3) as upool,
            tc.tile_pool(name="o", bufs=3) as opool,
            tc.tile_pool(name="praw", bufs=2, space="PSUM") as praw,
            tc.tile_pool(name="pout", bufs=2, space="PSUM") as pout,
            tc.tile_pool(name="pfc", bufs=2, space="PSUM") as pfc,
            tc.tile_pool(name="dram", bufs=1, space="DRAM") as dpool,
        ):
            cn = {}
            for cv, (sa, sb) in ((1, (SA1, SB1)), (2, (SA2, SB2))):
                cn[f"colA{cv}"] = cpool.tile([P, sa], gdt, name=f"colA{cv}t")
                cn[f"nrmA{cv}"] = cpool.tile([P, sa], f32, name=f"nrmA{cv}t")
                cn[f"colB{cv}"] = cpool.tile([P, sb], gdt, name=f"colB{cv}t")
                cn[f"nrmB{cv}"] = cpool.tile([P, sb], f32, name=f"nrmB{cv}t")
            Wa = cpool.tile([F, F], f32)
            Wb = cpool.tile([F, F], f32)
            ba = cpool.tile([1, F], f32)
            bb = cpool.tile([1, F], f32)
            wfc = cpool.tile([F, 1], f32)
            bfc = cpool.tile([1, 1], f32)
            iota = cpool.tile([P, P], gdt)
            ones_row = cpool.tile([1, P], f32)
            fc_row = cpool.tile([1, NPAIR], f32)

            loads = [(Wa, Wa_d), (Wb, Wb_d), (ba, ba_d), (bb, bb_d),
                     (wfc, wfc_d), (bfc, bfc_d), (iota, iota_d)]
            loads += [(cn[k], decl[k]) for k in cn]
            for sb_, dr in loads:
                nc.sync.dma_start(sb_[:], dr[:])
            nc.vector.memset(ones_row[:], 1.0)

            o1loc = dpool.tile([DPC, F], gdt)
            o1full = dpool.tile([N, F], gdt)

            def conv(lo_ap, hi_ap, W, bias, cv, nA, nB):
                is_conv2 = cv == 2
                st = {
                    0: dict(idxd=decl[f"idxA{cv}"], col=cn[f"colA{cv}"], nrm=cn[f"nrmA{cv}"],
                            S=int(nA.sum()), emitted=0, tiles={}),
                    1: dict(idxd=decl[f"idxB{cv}"], col=cn[f"colB{cv}"], nrm=cn[f"nrmB{cv}"],
                            S=int(nB.sum()), emitted=0, tiles={}),
                }
                src_ap = {0: lo_ap, 1: hi_ap}
                cursor = [0, 0]

                def ensure_group(stream, g):
                    s = st[stream]
                    while s["emitted"] <= g:
                        ge = s["emitted"]
                        gg = min(GCH, s["S"] - ge * GCH)
                        it = ipool.tile([P, GCH * 8], mybir.dt.int16, tag="idxs",
                                        name=f"it{cv}_{stream}_{ge}")
                        nc.sync.dma_start(
                            it[:, : gg * 8],
                            s["idxd"][:, ge * GCH * 8 : ge * GCH * 8 + gg * 8],
                        )
                        gt = gpool.tile([P, GCH, P], gdt, tag="gbuf",
                                        name=f"gb{cv}_{stream}_{ge}")
                        nc.gpsimd.dma_gather(
                            gt[:, :gg, :],
                            src_ap[stream],
                            it[:, : gg * 8],
                            gg * P,
                            gg * P,
                            F,
                            single_packet=False,
                            queue_num=(2 * stream + ge) % 4,
                        )
                        s["tiles"][ge] = gt
                        if ge - 6 in s["tiles"]:
                            del s["tiles"][ge - 6]
                        s["emitted"] += 1

                for t in range(NT):
                    dv = P if t < NT - 1 else LAST_DV
                    nchunks = int(nA[t] + nB[t])
                    acc = praw.tile([P, P], f32, tag="praw", name=f"acc{cv}_{t}")
                    done = 0
                    for stream in (0, 1):
                        nprog = nA if stream == 0 else nB
                        s = st[stream]
                        for k in range(int(nprog[t])):
                            c = cursor[stream] + k
                            g, slot = divmod(c, GCH)
                            ensure_group(stream, g)
                            oh = lpool.tile([P, P], gdt, tag="lhsT", name=f"oh{cv}_{t}_{stream}_{k}")
                            nc.vector.tensor_tensor(
                                out=oh[:],
                                in0=iota[:],
                                in1=s["col"][:, c : c + 1].to_broadcast([P, P]),
                                op=mybir.AluOpType.is_equal,
                            )
                            ohs = lpool.tile([P, P], gdt, tag="ohs", name=f"os{cv}_{t}_{stream}_{k}")
                            nc.scalar.activation(
                                ohs[:], oh[:], mybir.ActivationFunctionType.Copy,
                                scale=s["nrm"][:, c : c + 1],
                            )
                            nc.tensor.matmul(
                                acc[:],
                                s["tiles"][g][:, slot, :],
                                ohs[:],
                                start=(done == 0),
                                stop=(done == nchunks - 1),
                            )
                            done += 1
                        cursor[stream] += int(nprog[t])

                    u = upool.tile([P, P], f32, tag="u", name=f"u{cv}_{t}")
                    nc.vector.tensor_copy(u[:], acc[:])
                    vout = pout.tile([P, P], f32, tag="pout", name=f"v{cv}_{t}")
                    if not is_conv2:
                        # node-major out1[d, fo] = u.T @ Wa + 1 (x) ba
                        nc.tensor.matmul(vout[:dv, :], u[:, :dv], W[:], start=True, stop=False)
                        nc.tensor.matmul(vout[:dv, :], ones_row[:, :dv], bias[:], start=False, stop=True)
                        o = opool.tile([P, F], gdt, tag="o", name=f"o1_{t}")
                        nc.vector.tensor_copy(o[:dv, :], vout[:dv, :])
                        nc.sync.dma_start(o1loc[:][t * P : t * P + dv, :], o[:dv, :])
                        for k in range(4):
                            if t * P + dv == AGCH[k + 1]:
                                b0, b1 = AGCH[k], AGCH[k + 1]
                                nc.gpsimd.collective_compute(
                                    "AllGather",
                                    mybir.AluOpType.bypass,
                                    replica_groups=[[0, 1, 2, 3], [4, 5, 6, 7]],
                                    ins=[o1loc[:][b0:b1, :]],
                                    outs=[o1full[:][CPT * b0 : CPT * b1, :]],
                                )
                    else:
                        # feat-major v2[fo, d] = Wb.T @ u + bb (x) 1
                        nc.tensor.matmul(vout[:, :dv], W[:], u[:, :dv], start=True, stop=False)
                        nc.tensor.matmul(vout[:, :dv], bias[:], ones_row[:, :dv], start=False, stop=True)
                        sb2 = opool.tile([P, P], f32, tag="o2", name=f"s2_{t}")
                        nc.vector.tensor_copy(sb2[:, :dv], vout[:, :dv])
                        cvw = dv // 2
                        pooled = opool.tile([P, P // 2], f32, tag="pool", name=f"pl_{t}")
                        nc.vector.tensor_tensor(
                            out=pooled[:, :cvw],
                            in0=sb2[:, 0:dv:2],
                            in1=sb2[:, 1:dv:2],
                            op=mybir.AluOpType.max,
                        )
                        fcp = pfc.tile([1, P // 2], f32, tag="pfc", name=f"fc_{t}")
                        nc.tensor.matmul(fcp[:1, :cvw], wfc[:], pooled[:, :cvw], start=True, stop=True)
                        nc.vector.tensor_copy(fc_row[:, t * (P // 2) : t * (P // 2) + cvw], fcp[:1, :cvw])

            conv(x_lo[:], x_hi[:], Wa, ba, 1, nA1, nB1)
            full = o1full[:]
            conv(full[0:SPLIT, :], full[SPLIT:N, :], Wb, bb, 2, nA2, nB2)

            nc.scalar.activation(
                fc_row[:], fc_row[:], mybir.ActivationFunctionType.Sigmoid,
                bias=bfc[:1, :1], scale=1.0,
            )
            nc.sync.dma_start(out_d[:], fc_row[:])

    nc.compile()
    return nc


# ---------------------------------------------------------------- driver
def _cast_table(x):
    if BF16:
        import ml_dtypes

        return np.asarray(x).astype(ml_dtypes.bfloat16)
    return np.asarray(x, np.float32)


def kernel(**inputs):
    global LAST_EXEC_NS

    x1 = np.asarray(inputs["x1"], np.float32)
    x2 = np.asarray(inputs["x2"], np.float32)
    towers = []
    for xi, ei, ew, wa, wb, b_a, b_b in (
        (x1, "edge_index1", "edge_weight1", "W1a", "W1b", "b1a", "b1b"),
        (x2, "edge_index2", "edge_weight2", "W2a", "W2b", "b2a", "b2b"),
    ):
        cores = _edges_by_core(np.asarray(inputs[ei]), inputs[ew])
        perms = [_balance_pairs(c[1]) for c in cores]
        towers.append(
            dict(
                x=xi, cores=cores, perms=perms,
                Wa=np.asarray(inputs[wa], np.float32),
                Wb=np.asarray(inputs[wb], np.float32),
                ba=np.asarray(inputs[b_a], np.float32),
                bb=np.asarray(inputs[b_b], np.float32),
                fin=xi.shape[1],
            )
        )

    # conv2 source remap: global node -> chunk-major allgather position
    bnd = np.asarray(AGCH)
    for tw in towers:
        pos = np.zeros(N, np.int64)
        for q in range(CPT):
            L = _pos_map(*tw["perms"][q])
            k = np.searchsorted(bnd, L, side="right") - 1
            b0, b1 = bnd[k], bnd[k + 1]
            pos[q * DPC : (q + 1) * DPC] = CPT * b0 + q * (b1 - b0) + (L - b0)
        tw["pos"] = pos

    for tw in towers:
        tw["bk1"] = [
            _bucket_edges(r, l, n, *tw["perms"][q], None, SPLIT)
            for q, (r, l, n) in enumerate(tw["cores"])
        ]
        tw["bk2"] = [
            _bucket_edges(r, l, n, *tw["perms"][q], tw["pos"], SPLIT)
            for q, (r, l, n) in enumerate(tw["cores"])
        ]

    # program-uniform chunk counts: max over all 8 cores, per conv
    shape = []
    for bk in ("bk1", "bk2"):
        nA = np.zeros(NT, np.int64)
        nB = np.zeros(NT, np.int64)
        for tw in towers:
            for core in tw[bk]:
                cnt = core["cnt"].reshape(NT, 2)
                nA = np.maximum(nA, (cnt[:, 0] + P - 1) // P)
                nB = np.maximum(nB, (cnt[:, 1] + P - 1) // P)
        shape += [nA, nB]
    nA1, nB1, nA2, nB2 = shape

    iota = np.broadcast_to(np.arange(P, dtype=np.float32), (P, P)).copy()
    wfc = np.asarray(inputs["Wfc"], np.float32).reshape(F, 1)
    bfc = np.asarray(inputs["bfc"], np.float32).reshape(1, 1)

    in_maps = []
    for cid in range(NCORES):
        tw = towers[cid // CPT]
        q = cid % CPT
        fin = tw["fin"]
        Wa = np.zeros((F, F), np.float32)
        Wa[:fin, :] = tw["Wa"]
        xpad = _pad_rows(tw["x"], N, F)
        m = {
            "x_lo": _cast_table(xpad[:SPLIT]),
            "x_hi": _cast_table(xpad[SPLIT:]),
            "Wa": Wa,
            "Wb": tw["Wb"].astype(np.float32),
            "ba": tw["ba"].reshape(1, F).astype(np.float32),
            "bb": tw["bb"].reshape(1, F).astype(np.float32),
            "wfc": wfc,
            "bfc": bfc,
            "iota": _cast_table(iota),
        }
        for cv, bk, (nA, nB) in ((1, "bk1", (nA1, nB1)), (2, "bk2", (nA2, nB2))):
            (srcA, colA, nrmA), (srcB, colB, nrmB) = _pack_core(tw[bk][q], nA, nB)
            m[f"idxA{cv}"] = _wrap_idx(srcA, int(nA.sum()))
            m[f"idxB{cv}"] = _wrap_idx(srcB, int(nB.sum()))
            m[f"colA{cv}"] = _cast_table(colA)
            m[f"nrmA{cv}"] = nrmA
            m[f"colB{cv}"] = _cast_table(colB)
            m[f"nrmB{cv}"] = nrmB
        in_maps.append(m)

    nc = _build_program(shape)

    trace = bool(int(os.environ.get("KERNEL_TRACE", "0")))
    if trace:
        _install_trace_shim()
    res = run_bass_kernel_spmd(nc, in_maps, list(range(NCORES)), trace=trace)
    LAST_EXEC_NS = res.exec_time_ns

    # un-permute: cluster pair pr sits at fc position tile*64 + slot
    parts = []
    for cid in range(NCORES):
        tw = towers[cid // CPT]
        tile_of, slot_of = tw["perms"][cid % CPT]
        fc = res.results[cid]["outv"].reshape(-1)
        parts.append(fc[tile_of.astype(np.int64) * 64 + slot_of])
    return np.concatenate(parts).reshape(N, 1).astype(np.float32)


def _install_trace_shim():
    """Provide antenv.axon_hooks (absent in this image) so
    run_bass_kernel_spmd(trace=True) can drive NTFF profiling, and stub the
    artifact upload."""
    import contextlib
    import ctypes
    import types

    import concourse.bass_utils as bu

    bu.upload_artifacts = lambda tmpdir: ""

    so_path = "/opt/axon/libaxon_pjrt.so"
    lib = ctypes.CDLL(so_path)
    if not hasattr(lib, "axon_start_nrt_profile"):
        return
    lib.axon_start_nrt_profile.argtypes = [ctypes.POINTER(ctypes.c_int64), ctypes.c_size_t]
    lib.axon_start_nrt_profile.restype = ctypes.c_int64
    lib.axon_stop_nrt_profile.argtypes = [ctypes.c_char_p]
    lib.axon_stop_nrt_profile.restype = ctypes.c_int64

    @contextlib.contextmanager
    def _hook(output_dir, device_ids):
        import jax

        jax.devices()
        if device_ids:
            ids = (ctypes.c_int64 * len(device_ids))(*device_ids)
            rc = lib.axon_start_nrt_profile(ids, len(device_ids))
        else:
            rc = lib.axon_start_nrt_profile(None, 0)
        if rc != 0:
            raise RuntimeError(f"axon_start_nrt_profile rc={rc}")
        try:
            yield
        finally:
            n = lib.axon_stop_nrt_profile(str(output_dir).encode())
            print(f"ntff profile: {n} file(s) -> {output_dir}")

    mod = types.ModuleType("antenv.axon_hooks")
    mod.get_axon_ntff_profile_hook = lambda: _hook
    mod.set_axon_ntff_profile_hook = lambda h: None
    sys.modules["antenv.axon_hooks"] = mod
